# revision 26
# baseline (speedup 1.0000x reference)
"""GAT (2-layer, multi-head graph attention) Trainium2 kernel.

Contract: kernel(**inputs) takes the FULL unsharded inputs of
nn_GAT_7421703487704 and returns the full output (tuple matching the
reference: (relu(x), relu(x[:, 0, :]))).

Sharding: data-parallel over batch B=8 -> one graph per NeuronCore (8 cores).
Weights replicated. All shapes hardcoded.

Per-core layout strategy ("transposed" dataflow):
  - Host pre-transposes X -> XT [256,1024] and ships ladjT = (adj^T-1)*88
    (bf16) so the adjacency mask folds additively into attention logits
    pre-exp: exp(lrelu(f1_i+f2_j) + ladj) == adj * exp(lrelu(...)) to ~1e-7.
  - a_att / a_out are folded into the projection matmuls on the host
    (extra output columns f1,f2 / g1,g2).
  - Attention scores are built per [128,1024] tile of P^T [j,i]: the
    lrelu(ladjT + f2_j + f1bcast) runs either as one fused custom-DVE op
    or as two stock scalar_tensor_tensor ops on GpSimd (load balance knob),
    then one ACT Exp.
  - Row-normalisation sums come free as a ones-column in the aggregation
    matmul (stationary [h|1]); softmax division by row-sum is applied
    post-matmul at [64,1024] granularity.
  - Row vectors (f1/g1 rows, 1/rowsum, LN scale/bias rows) are broadcast
    across partitions by bouncing through internal DRAM and re-reading
    with a partition-step-0 access pattern - pure DMA-engine work.
  - LayerNorm runs in transposed layout via ones-matmul column sums;
    rstd uses exp(-0.5*ln(var+eps)) to stay in the exp/ln ACT table set.
"""

import sys

sys.path.insert(0, "/opt/trn_rl_repo")

import numpy as np
import ml_dtypes

import concourse.bass as bass
import concourse.mybir as mybir
import concourse.tile as tile
from concourse import bacc
from concourse.bass import ts
from concourse.bass_utils import run_bass_kernel_spmd
from concourse.masks import make_identity
import concourse.dve_ops as dops
from concourse.dve_ops import DveOp
from concourse.dve_spec import (
    Spec, Src0, Src1, C0, C1, Zero, One, maxx, minn, relu, sq,
    lower as dve_lower, _has_src1,
)
from concourse.dve_uop import DveOpSpec

F32 = mybir.dt.float32
BF16 = mybir.dt.bfloat16
AF = mybir.ActivationFunctionType
ALU = mybir.AluOpType
BF = ml_dtypes.bfloat16

B, N, NFEAT, NHID, NHEADS, NCLASS, NLAYERS = 8, 1024, 256, 64, 8, 256, 2
ALPHA = 0.2
OUTER_SLOPE = 0.01
LN_EPS = 1e-5
MASK_NEG = 88.0  # exp(-88) == 0 in f32; additive mask magnitude
NT = N // 128    # 8 node tiles
NJ = N // 128    # 8 j tiles

# Per-(head,jt) S-tile engine split: tiles with (h*NJ+jt) % GP_MOD < GP_CUT
# run the add+lrelu on GpSimd (2 stock stt ops); the rest use the fused
# custom-DVE op. Tuned from profile engine-occupancy.
GP_CUT = 0
GP_MOD = 9


# --------------------------------------------------------------------------
# custom DVE ops
# --------------------------------------------------------------------------

def _register_op(name, spec, subdim=False):
    if name in dops._SUB_OPCODE_FOR_NAME:
        return next(o for o in dops.OPS if o.name == name)
    row = dops._CUSTOM_DVE_ROW_BASE + len(dops.OPS)
    shas = {}
    for ver in ("v3", "v4"):
        try:
            s = DveOpSpec(name=name, opcode=row, uops=dve_lower(spec, ver=ver),
                          rd1_en=_has_src1(spec))
            shas[ver] = s.sha(ver)
        except Exception:
            pass
    op = DveOp(name, spec, subdim=subdim, uops_sha=shas)
    dops.OPS.append(op)
    dops.CUSTOM_DVE_SPECS[name] = spec
    dops._SUB_OPCODE_FOR_NAME[name] = row
    return op


def _make_ops():
    # t = lrelu((Src0 + C0) + Src1, alpha=C1)
    x = (Src0 + C0) + Src1
    lrelu_add3 = _register_op("GAT_LRELU_ADD3", Spec(
        body=maxx(x, x * C1),
        reference=lambda in0, in1, s0, s1: np.maximum(
            (in0 + s0) + in1, ((in0 + s0) + in1) * s1),
    ))
    # y = relu(Src0) + C0*min(Src1,1) - C0   (== lrelu_C0(elu(z)) given
    # Src0=z, Src1=exp(z); C0=1 gives plain elu(z))
    elu_y = _register_op("GAT_ELU_Y", Spec(
        body=relu(Src0) + minn(Src1, One) * C0 - C0,
        reference=lambda in0, in1, s0, s1: np.maximum(in0, 0)
        + np.minimum(in1, 1.0) * s0 - s0,
    ))
    # var+eps = Src0*C0 - Src1^2 + C1
    var_row = _register_op("GAT_VAR_ROW", Spec(
        body=Src0 * C0 - sq(Src1) + C1,
        reference=lambda in0, in1, s0, s1: in0 * s0 - in1 * in1 + s1,
    ))
    # out = Zero - Src0*Src1
    negmul = _register_op("GAT_NEGMUL", Spec(
        body=Zero - Src0 * Src1,
        reference=lambda in0, in1, s0, s1: -(in0 * in1),
    ))
    return lrelu_add3, elu_y, var_row, negmul


# --------------------------------------------------------------------------
# device kernel
# --------------------------------------------------------------------------

def build_kernel():
    LRELU_ADD3, ELU_Y, VAR_ROW, NEGMUL = _make_ops()

    nc = bacc.Bacc("TRN2", target_bir_lowering=False, debug=False)
    d = {
        "xt": nc.dram_tensor("xt", [2, 128, N], F32, kind="ExternalInput"),
        "ladjt": nc.dram_tensor("ladjt", [NJ, 128, N], BF16, kind="ExternalInput"),
        "maskrow": nc.dram_tensor("maskrow", [1, N], F32, kind="ExternalInput"),
        "m1rhs": nc.dram_tensor("m1rhs", [2, 128, 528], F32, kind="ExternalInput"),
        "wout": nc.dram_tensor("wout", [NHEADS, 64, 258], BF16, kind="ExternalInput"),
        "lnwb": nc.dram_tensor("lnwb", [128, 4], F32, kind="ExternalInput"),
        "out": nc.dram_tensor("xt_out", [2, 128, N], F32, kind="ExternalOutput"),
    }
    with tile.TileContext(nc) as tc:
        _build_body(nc, tc, d, LRELU_ADD3, ELU_Y, VAR_ROW, NEGMUL)
    nc.compile()
    return nc


def _bcast_src(row_ap, nparts):
    """Partition-step-0 AP replicating a DRAM row across nparts partitions."""
    return bass.AP(tensor=row_ap.tensor, offset=row_ap.offset,
                   ap=[[0, nparts]] + row_ap.ap[1:])


def _build_body(nc, tc, d, LRELU_ADD3, ELU_Y, VAR_ROW, NEGMUL):
    from contextlib import ExitStack
    ctx = ExitStack()
    P = 128

    const = ctx.enter_context(tc.tile_pool(name="const", bufs=1))
    work = ctx.enter_context(tc.tile_pool(name="work", bufs=1))
    dram = ctx.enter_context(tc.tile_pool(name="dram", bufs=1, space="DRAM"))

    # ---- persistent SBUF state ----
    ladjt = const.tile([P, NJ, N], BF16)          # 16KB/part
    for jt in range(NJ):
        nc.sync.dma_start(out=ladjt[:, jt, :], in_=d["ladjt"][jt])
    m1rhs = const.tile([P, 2, 528], F32)
    for kt in range(2):
        nc.sync.dma_start(out=m1rhs[:, kt, :], in_=d["m1rhs"][kt])
    wout = const.tile([64, NHEADS, 258], BF16)
    for h in range(NHEADS):
        nc.sync.dma_start(out=wout[:, h, :], in_=d["wout"][h])
    lnwb = const.tile([P, 4], F32)
    nc.sync.dma_start(out=lnwb, in_=d["lnwb"][:, :])
    maskrow = const.tile([1, N], F32)
    nc.sync.dma_start(out=maskrow, in_=d["maskrow"][:, :])
    ident = const.tile([P, P], F32)
    make_identity(nc, ident)
    ones_bf = const.tile([P, 1], BF16)
    nc.vector.memset(ones_bf, 1.0)

    xt32 = work.tile([P, 2, N], F32)              # residual state (f32)
    for ct in range(2):
        nc.sync.dma_start(out=xt32[:, ct, :], in_=d["xt"][ct])

    # h-augmented: per head [64 cols | ones]; stationary slices [128, 65]
    haug = work.tile([P, NT, 8 * 65], BF16)
    nc.vector.memset(haug, 1.0)  # ones columns at offsets h*65+64 persist
    hoaug = work.tile([P, NT, 257], BF16)         # [256 | ones]
    nc.vector.memset(hoaug[:, :, 256:257], 1.0)
    fcols = work.tile([P, NT, 16], F32)           # [f1(8) | f2(8)] per n-tile
    gcols = work.tile([P, NT, 2], F32)            # [g1 | g2] per n-tile
    yt = work.tile([64, NHEADS, N], BF16)         # y^T feature-blocks (K=64)
    f1row = work.tile([NHEADS, N], BF16)
    g1row = work.tile([1, N], BF16)
    znew = work.tile([P, 2, N], F32)              # pre-LN residual sum
    zb16 = work.tile([P, 2, N], BF16)
    sqb = work.tile([P, 2, N], BF16)

    # DRAM bounce rows for partition broadcasts
    dr_f1 = dram.tile([NHEADS, N], BF16)
    dr_r = dram.tile([NHEADS, N], F32)
    dr_g1 = dram.tile([1, N], BF16)
    dr_ro = dram.tile([1, N], F32)
    dr_ab = dram.tile([2, N], F32)

    spool = ctx.enter_context(tc.tile_pool(name="spool", bufs=4))
    ppool = ctx.enter_context(tc.tile_pool(name="ppool", bufs=3))
    f1bpool = ctx.enter_context(tc.tile_pool(name="f1b", bufs=2))
    epool = ctx.enter_context(tc.tile_pool(name="epool", bufs=2))
    epool1 = ctx.enter_context(tc.tile_pool(name="epool1", bufs=1))
    lnrows = ctx.enter_context(tc.tile_pool(name="lnrows", bufs=1))
    rowpool = ctx.enter_context(tc.tile_pool(name="rows", bufs=2))
    bigb = ctx.enter_context(tc.tile_pool(name="bigb", bufs=1))

    # Single PSUM pool: 4 rotating slots x 2 banks (4KB/part) = all 8 banks.
    psum = ctx.enter_context(tc.tile_pool(name="ps", bufs=4, space="PSUM"))

    HALF = [slice(0, 512), slice(512, 1024)]

    for layer in range(NLAYERS):
        # ================= M1: [h|f1|f2] = X @ [Wcat|w1|w2] (fp32) ==========
        for nt in range(NT):
            pa = psum.tile([P, 528], F32, tag="ps")
            for kt in range(2):
                nc.tensor.matmul(pa[:, 0:512], lhsT=xt32[:, kt, ts(nt, 128)],
                                 rhs=m1rhs[:, kt, 0:512],
                                 start=(kt == 0), stop=(kt == 1))
                nc.tensor.matmul(pa[:, 512:528], lhsT=xt32[:, kt, ts(nt, 128)],
                                 rhs=m1rhs[:, kt, 512:528],
                                 start=(kt == 0), stop=(kt == 1))
            nc.vector.tensor_copy(
                out=haug[:, nt, :].rearrange("p (h c) -> p h c", c=65)[:, :, 0:64],
                in_=pa[:, 0:512].rearrange("p (h c) -> p h c", c=64))
            nc.vector.tensor_copy(out=fcols[:, nt, :], in_=pa[:, 512:528])

        # ================= f1 rows -> DRAM bounce =================
        pf = psum.tile([8, N], F32, tag="ps")
        for nt in range(NT):
            nc.tensor.transpose(pf[:, ts(nt, 128)], fcols[:, nt, 0:8], ident)
        nc.vector.tensor_copy(out=f1row, in_=pf)

        # ================= inner attention, per head =================
        for h in range(NHEADS):
            f1sh = rowpool.tile([1, N], BF16, tag="f1sh")
            nc.sync.dma_start(out=f1sh, in_=f1row[h:h + 1, :])
            f1b = f1bpool.tile([P, N], BF16, tag="f1b")
            nc.gpsimd.partition_broadcast(out_ap=f1b, in_ap=f1sh)
            ph = psum.tile([65, N], F32, tag="ps")
            for jt in range(NJ):
                f2c = fcols[:, jt, 8 + h:9 + h]
                p_ = ppool.tile([P, N], BF16, tag="p")
                if (h * NJ + jt) % GP_MOD < GP_CUT:
                    ta = spool.tile([P, N], BF16, tag="ta")
                    nc.gpsimd.tensor_add(ta, ladjt[:, jt, :], f1b)
                    tb = spool.tile([P, N], BF16, tag="tb")
                    nc.scalar.activation(out=tb, in_=ta, func=AF.Lrelu,
                                         bias=f2c, alpha=ALPHA)
                    nc.scalar.activation(out=p_, in_=tb, func=AF.Exp)
                else:
                    t0 = spool.tile([P, N], BF16, tag="t")
                    nc.vector._custom_dve(LRELU_ADD3, out=t0,
                                          in0=ladjt[:, jt, :], in1=f1b,
                                          s0=f2c, s1=ALPHA)
                    nc.scalar.activation(out=p_, in_=t0, func=AF.Exp)
                for s in range(2):
                    nc.tensor.matmul(ph[:, HALF[s]],
                                     lhsT=haug[:, jt, h * 65:h * 65 + 65],
                                     rhs=p_[:, HALF[s]],
                                     start=(jt == 0), stop=(jt == NJ - 1))
            # epilogue: y^T[h] = lrelu01(elu(hp / r))
            rsb = rowpool.tile([65, N], F32, tag="rsb")
            nc.scalar.activation(out=rsb[64:65, :], in_=ph[64:65, :],
                                 func=AF.Identity)
            rr0 = rowpool.tile([1, N], F32, tag="rr0")
            nc.sync.dma_start(out=rr0, in_=rsb[64:65, :])
            rinv = rowpool.tile([1, N], F32, tag="rinv")
            nc.vector.reciprocal_approx_fast(out=rinv, in_=rr0)
            rb = epool.tile([64, N], F32, tag="rb")
            nc.gpsimd.partition_broadcast(out_ap=rb, in_ap=rinv)
            z = epool.tile([64, N], BF16, tag="z")
            nc.vector.tensor_mul(z, ph[0:64, :], rb)
            e = epool.tile([64, N], BF16, tag="e")
            nc.scalar.activation(out=e, in_=z, func=AF.Exp)
            nc.vector._custom_dve(ELU_Y, out=yt[:, h, :], in0=z, in1=e,
                                  s0=OUTER_SLOPE)

        # ================= M3: [ho|g1|g2] = y @ [W_out|b1|b2] ===============
        for nt in range(NT):
            pa = psum.tile([P, 258], F32, tag="ps")
            for h in range(NHEADS):
                nc.tensor.matmul(pa[:, 0:258], lhsT=yt[:, h, ts(nt, 128)],
                                 rhs=wout[:, h, 0:258],
                                 start=(h == 0), stop=(h == NHEADS - 1))
            nc.vector.tensor_copy(out=hoaug[:, nt, 0:256], in_=pa[:, 0:256])
            nc.vector.tensor_copy(out=gcols[:, nt, :], in_=pa[:, 256:258])

        # ================= g1 row =================
        pf2 = psum.tile([2, N], F32, tag="ps")
        for nt in range(NT):
            nc.tensor.transpose(pf2[:, ts(nt, 128)], gcols[:, nt, 0:2], ident)
        nc.vector.tensor_copy(out=g1row, in_=pf2[0:1, :])
        g1b = f1bpool.tile([P, N], BF16, tag="f1b")
        nc.gpsimd.partition_broadcast(out_ap=g1b, in_ap=g1row)

        # ================= outer attention =================
        pu0 = psum.tile([P, N], F32, tag="ps")
        pu1 = psum.tile([P, N], F32, tag="ps")
        pro = psum.tile([1, N], F32, tag="ps")
        for jt in range(NJ):
            g2c = gcols[:, jt, 1:2]
            t0 = spool.tile([P, N], BF16, tag="t")
            nc.vector._custom_dve(LRELU_ADD3, out=t0, in0=ladjt[:, jt, :],
                                  in1=g1b, s0=g2c, s1=ALPHA)
            po = ppool.tile([P, N], BF16, tag="p")
            nc.scalar.activation(out=po, in_=t0, func=AF.Exp)
            for s in range(2):
                nc.tensor.matmul(pu0[:, HALF[s]], lhsT=hoaug[:, jt, 0:128],
                                 rhs=po[:, HALF[s]],
                                 start=(jt == 0), stop=(jt == NJ - 1))
                nc.tensor.matmul(pu1[:, HALF[s]], lhsT=hoaug[:, jt, 128:256],
                                 rhs=po[:, HALF[s]],
                                 start=(jt == 0), stop=(jt == NJ - 1))
                nc.tensor.matmul(pro[:, HALF[s]], lhsT=hoaug[:, jt, 256:257],
                                 rhs=po[:, HALF[s]],
                                 start=(jt == 0), stop=(jt == NJ - 1))
        # z = uoT / ro ; znew = xt + elu(z)
        roinv = rowpool.tile([1, N], F32, tag="roinv")
        nc.vector.reciprocal_approx_fast(out=roinv, in_=pro[0:1, :])
        rob = bigb.tile([P, N], F32, tag="rob")
        nc.gpsimd.partition_broadcast(out_ap=rob, in_ap=roinv)
        for ct, pu in enumerate((pu0, pu1)):
            zo = epool1.tile([P, N], BF16, tag="zo")
            nc.vector.tensor_mul(zo, pu, rob)
            eo = epool1.tile([P, N], BF16, tag="eo")
            nc.scalar.activation(out=eo, in_=zo, func=AF.Exp)
            t1 = epool1.tile([P, N], BF16, tag="t1o")
            nc.vector._custom_dve(ELU_Y, out=t1, in0=zo, in1=eo, s0=1.0)
            nc.gpsimd.tensor_add(znew[:, ct, :], xt32[:, ct, :], t1)

        # ================= masked LayerNorm (transposed) =================
        for ct in range(2):
            nc.gpsimd.tensor_copy(out=zb16[:, ct, :], in_=znew[:, ct, :])
            nc.gpsimd.tensor_mul(sqb[:, ct, :], zb16[:, ct, :], zb16[:, ct, :])
        pSs = psum.tile([1, N], F32, tag="ps")
        pSq = psum.tile([1, N], F32, tag="ps")
        for s in range(2):
            for ct in range(2):
                nc.tensor.matmul(pSs[:, HALF[s]], lhsT=ones_bf,
                                 rhs=zb16[:, ct, HALF[s]],
                                 start=(ct == 0), stop=(ct == 1))
                nc.tensor.matmul(pSq[:, HALF[s]], lhsT=ones_bf,
                                 rhs=sqb[:, ct, HALF[s]],
                                 start=(ct == 0), stop=(ct == 1))
        # rows: mu, var+eps, rstd = exp(-0.5*ln(var+eps)), A = rstd*mask, B=-mu*A
        r0 = lnrows.tile([1, N], F32, tag="r0")
        nc.vector.tensor_scalar_mul(r0, pSs[0:1, :], 1.0 / NFEAT)
        r1a = lnrows.tile([1, N], F32, tag="r1")
        nc.vector._custom_dve(VAR_ROW, out=r1a, in0=pSq[0:1, :], in1=r0,
                              s0=1.0 / NFEAT, s1=LN_EPS)
        r2 = lnrows.tile([1, N], F32, tag="r2")
        nc.scalar.activation(out=r2, in_=r1a, func=AF.Ln)
        r1b = lnrows.tile([1, N], F32, tag="r1")
        nc.scalar.activation(out=r1b, in_=r2, func=AF.Exp, scale=-0.5)
        arow = lnrows.tile([1, N], F32, tag="arow")
        nc.vector.tensor_mul(arow, r1b, maskrow)
        brow = lnrows.tile([1, N], F32, tag="brow")
        nc.vector._custom_dve(NEGMUL, out=brow, in0=r0, in1=arow)
        ab = bigb.tile([P, N], F32, tag="ab")
        nc.gpsimd.partition_broadcast(out_ap=ab, in_ap=arow)
        bb = bigb.tile([P, N], F32, tag="bb")
        nc.gpsimd.partition_broadcast(out_ap=bb, in_ap=brow)
        last = layer == NLAYERS - 1
        for ct in range(2):
            u1 = epool1.tile([P, N], F32, tag="u1")
            nc.gpsimd.tensor_mul(u1, znew[:, ct, :], ab)
            u2 = epool1.tile([P, N], F32, tag="u2")
            nc.gpsimd.tensor_add(u2, u1, bb)
            nc.scalar.activation(out=xt32[:, ct, :], in_=u2,
                                 func=(AF.Relu if last else AF.Identity),
                                 bias=lnwb[:, 2 + ct:3 + ct],
                                 scale=lnwb[:, ct:ct + 1])

    for ct in range(2):
        nc.sync.dma_start(out=d["out"][ct], in_=xt32[:, ct, :])
    ctx.close()


# --------------------------------------------------------------------------
# host wrapper
# --------------------------------------------------------------------------

_NC_CACHE = {}


def _get_nc():
    if "nc" not in _NC_CACHE:
        _NC_CACHE["nc"] = build_kernel()
    return _NC_CACHE["nc"]


def _prep_shared(W_att, a_att, W_out, a_out, ln_w, ln_b):
    # m1rhs = [Wcat(512) | w1(8) | w2(8)] : [256, 528]
    Wcat = np.transpose(W_att, (1, 0, 2)).reshape(NFEAT, NHEADS * NHID)
    w1 = np.einsum("hfo,ho->fh", W_att, a_att[:, :NHID])
    w2 = np.einsum("hfo,ho->fh", W_att, a_att[:, NHID:])
    m1 = np.concatenate([Wcat, w1, w2], axis=1).astype(np.float32)  # [256, 528]
    m1rhs = m1.reshape(2, 128, 528)
    # wout = [W_out | W_out@ao1 | W_out@ao2] : [512, 258] -> [8, 64, 258]
    b1 = W_out @ a_out[:NCLASS]
    b2 = W_out @ a_out[NCLASS:]
    wo = np.concatenate([W_out, b1[:, None], b2[:, None]], axis=1).astype(BF)
    wout = wo.reshape(NHEADS, 64, 258)
    lnwb = np.stack([ln_w[:128], ln_w[128:], ln_b[:128], ln_b[128:]],
                    axis=1).astype(np.float32)                  # [128, 4]
    return m1rhs, wout, lnwb


def kernel(x, adj, mask, W_att, a_att, W_out, a_out, ln_w, ln_b):
    x = np.asarray(x, np.float32)
    adj = np.asarray(adj)
    mask = np.asarray(mask)
    m1rhs, wout, lnwb = _prep_shared(
        np.asarray(W_att, np.float32), np.asarray(a_att, np.float32),
        np.asarray(W_out, np.float32), np.asarray(a_out, np.float32),
        np.asarray(ln_w, np.float32), np.asarray(ln_b, np.float32))

    in_maps = []
    for b in range(B):
        xt = np.ascontiguousarray(x[b].T).reshape(2, 128, N)
        adjt = (adj[b] > 0).astype(np.float32).T
        ladjt = ((adjt - 1.0) * MASK_NEG).astype(BF).reshape(NJ, 128, N)
        maskrow = (mask[b] != 0).astype(np.float32).reshape(1, N)
        in_maps.append({
            "xt": xt, "ladjt": np.ascontiguousarray(ladjt),
            "maskrow": maskrow, "m1rhs": m1rhs, "wout": wout, "lnwb": lnwb,
        })

    nc = _get_nc()
    res = run_bass_kernel_spmd(nc, in_maps, core_ids=list(range(B)))
    out = np.empty((B, N, NCLASS), np.float32)
    for b in range(B):
        xt_out = res.results[b]["xt_out"].reshape(NFEAT, N)
        out[b] = xt_out.T
    return out, out[:, 0, :]


if __name__ == "__main__":
    rng = np.random.default_rng(0)
    ins = {
        "x": rng.standard_normal((B, N, NFEAT)).astype(np.float32),
        "adj": (rng.random((B, N, N)) < 0.5).astype(np.int32),
        "mask": (rng.random((B, N)) < 0.5).astype(np.int32),
        "W_att": (rng.standard_normal((NHEADS, NFEAT, NHID)) * 0.05).astype(np.float32),
        "a_att": (rng.standard_normal((NHEADS, 2 * NHID)) * 0.05).astype(np.float32),
        "W_out": (rng.standard_normal((NHID * NHEADS, NCLASS)) * 0.05).astype(np.float32),
        "a_out": (rng.standard_normal(2 * NCLASS) * 0.05).astype(np.float32),
        "ln_w": np.ones(NCLASS, np.float32),
        "ln_b": np.zeros(NCLASS, np.float32),
    }
    o1, o2 = kernel(**ins)
    print("out", o1.shape, o1.dtype, float(np.abs(o1).max()))


# revision 27
# speedup vs baseline: 1.1378x; 1.1378x over previous
"""GAT (2-layer, multi-head graph attention) Trainium2 kernel.

Contract: kernel(**inputs) takes the FULL unsharded inputs of
nn_GAT_7421703487704 and returns the full output (tuple matching the
reference: (relu(x), relu(x[:, 0, :]))).

Sharding: data-parallel over batch B=8 -> one graph per NeuronCore (8 cores).
Weights replicated. All shapes hardcoded.

Per-core layout strategy ("transposed" dataflow):
  - Host pre-transposes X -> XT [256,1024] and ships ladjT = (adj^T-1)*88
    (bf16) so the adjacency mask folds additively into attention logits
    pre-exp: exp(lrelu(f1_i+f2_j) + ladj) == adj * exp(lrelu(...)) to ~1e-7.
  - a_att / a_out are folded into the projection matmuls on the host
    (extra output columns f1,f2 / g1,g2).
  - Attention scores are built per [128,1024] tile of P^T [j,i]: the
    lrelu(ladjT + f2_j + f1bcast) runs either as one fused custom-DVE op
    or as two stock scalar_tensor_tensor ops on GpSimd (load balance knob),
    then one ACT Exp.
  - Row-normalisation sums come free as a ones-column in the aggregation
    matmul (stationary [h|1]); softmax division by row-sum is applied
    post-matmul at [64,1024] granularity.
  - Row vectors (f1/g1 rows, 1/rowsum, LN scale/bias rows) are broadcast
    across partitions by bouncing through internal DRAM and re-reading
    with a partition-step-0 access pattern - pure DMA-engine work.
  - LayerNorm runs in transposed layout via ones-matmul column sums;
    rstd uses exp(-0.5*ln(var+eps)) to stay in the exp/ln ACT table set.
"""

import sys

sys.path.insert(0, "/opt/trn_rl_repo")

import numpy as np
import ml_dtypes

import concourse.bass as bass
import concourse.mybir as mybir
import concourse.tile as tile
from concourse import bacc
from concourse.bass import ts
from concourse.bass_utils import run_bass_kernel_spmd
from concourse.masks import make_identity
import concourse.dve_ops as dops
from concourse.dve_ops import DveOp
from concourse.dve_spec import (
    Spec, Src0, Src1, C0, C1, Zero, One, maxx, minn, relu, sq,
    lower as dve_lower, _has_src1,
)
from concourse.dve_uop import DveOpSpec

F32 = mybir.dt.float32
BF16 = mybir.dt.bfloat16
AF = mybir.ActivationFunctionType
ALU = mybir.AluOpType
BF = ml_dtypes.bfloat16

B, N, NFEAT, NHID, NHEADS, NCLASS, NLAYERS = 8, 1024, 256, 64, 8, 256, 2
ALPHA = 0.2
OUTER_SLOPE = 0.01
LN_EPS = 1e-5
MASK_NEG = 88.0  # exp(-88) == 0 in f32; additive mask magnitude
NT = N // 128    # 8 node tiles
NJ = N // 128    # 8 j tiles

# Per-(head,jt) S-tile engine split: tiles with (h*NJ+jt) % GP_MOD < GP_CUT
# run the add+lrelu on GpSimd (2 stock stt ops); the rest use the fused
# custom-DVE op. Tuned from profile engine-occupancy.
GP_CUT = 0
GP_MOD = 9


# --------------------------------------------------------------------------
# custom DVE ops
# --------------------------------------------------------------------------

def _register_op(name, spec, subdim=False):
    if name in dops._SUB_OPCODE_FOR_NAME:
        return next(o for o in dops.OPS if o.name == name)
    row = dops._CUSTOM_DVE_ROW_BASE + len(dops.OPS)
    shas = {}
    for ver in ("v3", "v4"):
        try:
            s = DveOpSpec(name=name, opcode=row, uops=dve_lower(spec, ver=ver),
                          rd1_en=_has_src1(spec))
            shas[ver] = s.sha(ver)
        except Exception:
            pass
    op = DveOp(name, spec, subdim=subdim, uops_sha=shas)
    dops.OPS.append(op)
    dops.CUSTOM_DVE_SPECS[name] = spec
    dops._SUB_OPCODE_FOR_NAME[name] = row
    return op


def _make_ops():
    # t = lrelu((Src0 + C0) + Src1, alpha=C1)
    x = (Src0 + C0) + Src1
    lrelu_add3 = _register_op("GAT_LRELU_ADD3", Spec(
        body=maxx(x, x * C1),
        reference=lambda in0, in1, s0, s1: np.maximum(
            (in0 + s0) + in1, ((in0 + s0) + in1) * s1),
    ))
    # y = relu(Src0) + C0*min(Src1,1) - C0   (== lrelu_C0(elu(z)) given
    # Src0=z, Src1=exp(z); C0=1 gives plain elu(z))
    elu_y = _register_op("GAT_ELU_Y", Spec(
        body=relu(Src0) + minn(Src1, One) * C0 - C0,
        reference=lambda in0, in1, s0, s1: np.maximum(in0, 0)
        + np.minimum(in1, 1.0) * s0 - s0,
    ))
    # var+eps = Src0*C0 - Src1^2 + C1
    var_row = _register_op("GAT_VAR_ROW", Spec(
        body=Src0 * C0 - sq(Src1) + C1,
        reference=lambda in0, in1, s0, s1: in0 * s0 - in1 * in1 + s1,
    ))
    # out = Zero - Src0*Src1
    negmul = _register_op("GAT_NEGMUL", Spec(
        body=Zero - Src0 * Src1,
        reference=lambda in0, in1, s0, s1: -(in0 * in1),
    ))
    return lrelu_add3, elu_y, var_row, negmul


# --------------------------------------------------------------------------
# device kernel
# --------------------------------------------------------------------------

def build_kernel():
    LRELU_ADD3, ELU_Y, VAR_ROW, NEGMUL = _make_ops()

    nc = bacc.Bacc("TRN2", target_bir_lowering=False, debug=False)
    d = {
        "xt": nc.dram_tensor("xt", [2, 128, N], F32, kind="ExternalInput"),
        "ladjt": nc.dram_tensor("ladjt", [NJ, 128, N], BF16, kind="ExternalInput"),
        "maskrow": nc.dram_tensor("maskrow", [1, N], F32, kind="ExternalInput"),
        "m1rhs": nc.dram_tensor("m1rhs", [2, 128, 528], F32, kind="ExternalInput"),
        "wout": nc.dram_tensor("wout", [NHEADS, 64, 258], BF16, kind="ExternalInput"),
        "lnwb": nc.dram_tensor("lnwb", [128, 4], F32, kind="ExternalInput"),
        "out": nc.dram_tensor("xt_out", [2, 128, N], F32, kind="ExternalOutput"),
    }
    with tile.TileContext(nc) as tc:
        _build_body(nc, tc, d, LRELU_ADD3, ELU_Y, VAR_ROW, NEGMUL)
    nc.compile()
    return nc


def _bcast_src(row_ap, nparts):
    """Partition-step-0 AP replicating a DRAM row across nparts partitions."""
    return bass.AP(tensor=row_ap.tensor, offset=row_ap.offset,
                   ap=[[0, nparts]] + row_ap.ap[1:])


def _build_body(nc, tc, d, LRELU_ADD3, ELU_Y, VAR_ROW, NEGMUL):
    from contextlib import ExitStack
    ctx = ExitStack()
    P = 128

    const = ctx.enter_context(tc.tile_pool(name="const", bufs=1))
    work = ctx.enter_context(tc.tile_pool(name="work", bufs=1))
    dram = ctx.enter_context(tc.tile_pool(name="dram", bufs=1, space="DRAM"))

    # ---- persistent SBUF state ----
    ladjt = const.tile([P, NJ, N], BF16)          # 16KB/part
    for jt in range(NJ):
        nc.sync.dma_start(out=ladjt[:, jt, :], in_=d["ladjt"][jt])
    m1rhs = const.tile([P, 2, 528], F32)
    for kt in range(2):
        nc.sync.dma_start(out=m1rhs[:, kt, :], in_=d["m1rhs"][kt])
    wout = const.tile([64, NHEADS, 258], BF16)
    for h in range(NHEADS):
        nc.sync.dma_start(out=wout[:, h, :], in_=d["wout"][h])
    lnwb = const.tile([P, 4], F32)
    nc.sync.dma_start(out=lnwb, in_=d["lnwb"][:, :])
    maskrow = const.tile([1, N], F32)
    nc.sync.dma_start(out=maskrow, in_=d["maskrow"][:, :])
    ident = const.tile([P, P], F32)
    make_identity(nc, ident)
    ones_bf = const.tile([P, 1], BF16)
    nc.vector.memset(ones_bf, 1.0)

    xt32 = work.tile([P, 2, N], F32)              # residual state (f32)
    for ct in range(2):
        nc.sync.dma_start(out=xt32[:, ct, :], in_=d["xt"][ct])

    # h-augmented: per head [64 cols | ones]; stationary slices [128, 65]
    haug = work.tile([P, NT, 8 * 65], BF16)
    nc.vector.memset(haug, 1.0)  # ones columns at offsets h*65+64 persist
    hoaug = work.tile([P, NT, 257], BF16)         # [256 | ones]
    nc.vector.memset(hoaug[:, :, 256:257], 1.0)
    fcols = work.tile([P, NT, 16], F32)           # [f1(8) | f2(8)] per n-tile
    gcols = work.tile([P, NT, 2], F32)            # [g1 | g2] per n-tile
    yt = work.tile([64, NHEADS, N], BF16)         # y^T feature-blocks (K=64)
    f1row = work.tile([NHEADS, N], BF16)
    g1row = work.tile([1, N], BF16)
    znew = work.tile([P, 2, N], F32)              # pre-LN residual sum
    zb16 = work.tile([P, 2, N], BF16)
    sqb = work.tile([P, 2, N], BF16)

    # DRAM bounce rows for partition broadcasts
    dr_f1 = dram.tile([NHEADS, N], BF16)
    dr_r = dram.tile([NHEADS, N], F32)
    dr_g1 = dram.tile([1, N], BF16)
    dr_ro = dram.tile([1, N], F32)
    dr_ab = dram.tile([2, N], F32)

    spool = ctx.enter_context(tc.tile_pool(name="spool", bufs=4))
    ppool = ctx.enter_context(tc.tile_pool(name="ppool", bufs=3))
    f1bpool = ctx.enter_context(tc.tile_pool(name="f1b", bufs=2))
    epool = ctx.enter_context(tc.tile_pool(name="epool", bufs=2))
    epool1 = ctx.enter_context(tc.tile_pool(name="epool1", bufs=1))
    lnrows = ctx.enter_context(tc.tile_pool(name="lnrows", bufs=1))
    rowpool = ctx.enter_context(tc.tile_pool(name="rows", bufs=2))
    bigb = ctx.enter_context(tc.tile_pool(name="bigb", bufs=1))

    # Single PSUM pool: 4 rotating slots x 2 banks (4KB/part) = all 8 banks.
    psum = ctx.enter_context(tc.tile_pool(name="ps", bufs=4, space="PSUM"))

    HALF = [slice(0, 512), slice(512, 1024)]

    for layer in range(NLAYERS):
        # ================= M1: [h|f1|f2] = X @ [Wcat|w1|w2] (fp32) ==========
        for nt in range(NT):
            pa = psum.tile([P, 528], F32, tag="ps")
            for kt in range(2):
                nc.tensor.matmul(pa[:, 0:512], lhsT=xt32[:, kt, ts(nt, 128)],
                                 rhs=m1rhs[:, kt, 0:512],
                                 start=(kt == 0), stop=(kt == 1))
                nc.tensor.matmul(pa[:, 512:528], lhsT=xt32[:, kt, ts(nt, 128)],
                                 rhs=m1rhs[:, kt, 512:528],
                                 start=(kt == 0), stop=(kt == 1))
            nc.vector.tensor_copy(
                out=haug[:, nt, :].rearrange("p (h c) -> p h c", c=65)[:, :, 0:64],
                in_=pa[:, 0:512].rearrange("p (h c) -> p h c", c=64))
            nc.vector.tensor_copy(out=fcols[:, nt, :], in_=pa[:, 512:528])

        # ================= f1 rows -> DRAM bounce =================
        pf = psum.tile([8, N], F32, tag="ps")
        for nt in range(NT):
            nc.tensor.transpose(pf[:, ts(nt, 128)], fcols[:, nt, 0:8], ident)
        nc.vector.tensor_copy(out=f1row, in_=pf)

        # ================= inner attention, per head =================
        for h in range(NHEADS):
            f1sh = rowpool.tile([1, N], BF16, tag="f1sh")
            nc.sync.dma_start(out=f1sh, in_=f1row[h:h + 1, :])
            f1b = f1bpool.tile([P, N], BF16, tag="f1b")
            nc.gpsimd.partition_broadcast(out_ap=f1b, in_ap=f1sh)
            ph = psum.tile([65, N], F32, tag="ps")
            for jt in range(NJ):
                f2c = fcols[:, jt, 8 + h:9 + h]
                p_ = ppool.tile([P, N], BF16, tag="p")
                if (h * NJ + jt) % GP_MOD < GP_CUT:
                    ta = spool.tile([P, N], BF16, tag="ta")
                    nc.gpsimd.tensor_add(ta, ladjt[:, jt, :], f1b)
                    tb = spool.tile([P, N], BF16, tag="tb")
                    nc.scalar.activation(out=tb, in_=ta, func=AF.Lrelu,
                                         bias=f2c, alpha=ALPHA)
                    nc.scalar.activation(out=p_, in_=tb, func=AF.Exp)
                else:
                    t0 = spool.tile([P, N], BF16, tag="t")
                    nc.vector._custom_dve(LRELU_ADD3, out=t0,
                                          in0=ladjt[:, jt, :], in1=f1b,
                                          s0=f2c, s1=ALPHA)
                    nc.scalar.activation(out=p_, in_=t0, func=AF.Exp)
                for s in range(2):
                    nc.tensor.matmul(ph[:, HALF[s]],
                                     lhsT=haug[:, jt, h * 65:h * 65 + 65],
                                     rhs=p_[:, HALF[s]],
                                     start=(jt == 0), stop=(jt == NJ - 1))
            # epilogue: y^T[h] = lrelu01(elu(hp / r))
            rsb = rowpool.tile([65, N], F32, tag="rsb")
            nc.scalar.activation(out=rsb[64:65, :], in_=ph[64:65, :],
                                 func=AF.Identity)
            rr0 = rowpool.tile([1, N], F32, tag="rr0")
            nc.sync.dma_start(out=rr0, in_=rsb[64:65, :])
            rinv = rowpool.tile([1, N], F32, tag="rinv")
            nc.vector.reciprocal_approx_fast(out=rinv, in_=rr0)
            rb = epool.tile([64, N], F32, tag="rb")
            nc.gpsimd.partition_broadcast(out_ap=rb, in_ap=rinv)
            z = epool.tile([64, N], BF16, tag="z")
            nc.vector.tensor_mul(z, ph[0:64, :], rb)
            e = epool.tile([64, N], BF16, tag="e")
            nc.scalar.activation(out=e, in_=z, func=AF.Exp)
            nc.vector._custom_dve(ELU_Y, out=yt[:, h, :], in0=z, in1=e,
                                  s0=OUTER_SLOPE)

        # ================= M3: [ho|g1|g2] = y @ [W_out|b1|b2] ===============
        for nt in range(NT):
            pa = psum.tile([P, 258], F32, tag="ps")
            for h in range(NHEADS):
                nc.tensor.matmul(pa[:, 0:258], lhsT=yt[:, h, ts(nt, 128)],
                                 rhs=wout[:, h, 0:258],
                                 start=(h == 0), stop=(h == NHEADS - 1))
            nc.vector.tensor_copy(out=hoaug[:, nt, 0:256], in_=pa[:, 0:256])
            nc.vector.tensor_copy(out=gcols[:, nt, :], in_=pa[:, 256:258])

        # ================= g1 row =================
        pf2 = psum.tile([2, N], F32, tag="ps")
        for nt in range(NT):
            nc.tensor.transpose(pf2[:, ts(nt, 128)], gcols[:, nt, 0:2], ident)
        nc.vector.tensor_copy(out=g1row, in_=pf2[0:1, :])
        g1b = f1bpool.tile([P, N], BF16, tag="f1b")
        nc.gpsimd.partition_broadcast(out_ap=g1b, in_ap=g1row)

        # ================= outer attention =================
        pu0 = psum.tile([P, N], F32, tag="ps")
        pu1 = psum.tile([P, N], F32, tag="ps")
        pro = psum.tile([1, N], F32, tag="ps")
        for jt in range(NJ):
            g2c = gcols[:, jt, 1:2]
            t0 = spool.tile([P, N], BF16, tag="t")
            nc.vector._custom_dve(LRELU_ADD3, out=t0, in0=ladjt[:, jt, :],
                                  in1=g1b, s0=g2c, s1=ALPHA)
            po = ppool.tile([P, N], BF16, tag="p")
            nc.scalar.activation(out=po, in_=t0, func=AF.Exp)
            for s in range(2):
                nc.tensor.matmul(pu0[:, HALF[s]], lhsT=hoaug[:, jt, 0:128],
                                 rhs=po[:, HALF[s]],
                                 start=(jt == 0), stop=(jt == NJ - 1))
                nc.tensor.matmul(pu1[:, HALF[s]], lhsT=hoaug[:, jt, 128:256],
                                 rhs=po[:, HALF[s]],
                                 start=(jt == 0), stop=(jt == NJ - 1))
                nc.tensor.matmul(pro[:, HALF[s]], lhsT=hoaug[:, jt, 256:257],
                                 rhs=po[:, HALF[s]],
                                 start=(jt == 0), stop=(jt == NJ - 1))
        # z = uoT / ro ; znew = xt + elu(z)
        roinv = rowpool.tile([1, N], F32, tag="roinv")
        nc.vector.reciprocal_approx_fast(out=roinv, in_=pro[0:1, :])
        rob = bigb.tile([P, N], F32, tag="rob")
        nc.gpsimd.partition_broadcast(out_ap=rob, in_ap=roinv)
        for ct, pu in enumerate((pu0, pu1)):
            zo = epool1.tile([P, N], BF16, tag="zo")
            nc.vector.tensor_mul(zo, pu, rob)
            eo = epool1.tile([P, N], BF16, tag="eo")
            nc.scalar.activation(out=eo, in_=zo, func=AF.Exp)
            t1 = epool1.tile([P, N], BF16, tag="t1o")
            nc.vector._custom_dve(ELU_Y, out=t1, in0=zo, in1=eo, s0=1.0)
            nc.vector.tensor_add(znew[:, ct, :], xt32[:, ct, :], t1)

        # ================= masked LayerNorm (transposed) =================
        for ct in range(2):
            nc.vector.tensor_copy(out=zb16[:, ct, :], in_=znew[:, ct, :])
            nc.vector.tensor_mul(sqb[:, ct, :], zb16[:, ct, :], zb16[:, ct, :])
        pSs = psum.tile([1, N], F32, tag="ps")
        pSq = psum.tile([1, N], F32, tag="ps")
        for s in range(2):
            for ct in range(2):
                nc.tensor.matmul(pSs[:, HALF[s]], lhsT=ones_bf,
                                 rhs=zb16[:, ct, HALF[s]],
                                 start=(ct == 0), stop=(ct == 1))
                nc.tensor.matmul(pSq[:, HALF[s]], lhsT=ones_bf,
                                 rhs=sqb[:, ct, HALF[s]],
                                 start=(ct == 0), stop=(ct == 1))
        # rows: mu, var+eps, rstd = exp(-0.5*ln(var+eps)), A = rstd*mask, B=-mu*A
        r0 = lnrows.tile([1, N], F32, tag="r0")
        nc.vector.tensor_scalar_mul(r0, pSs[0:1, :], 1.0 / NFEAT)
        r1a = lnrows.tile([1, N], F32, tag="r1")
        nc.vector._custom_dve(VAR_ROW, out=r1a, in0=pSq[0:1, :], in1=r0,
                              s0=1.0 / NFEAT, s1=LN_EPS)
        r2 = lnrows.tile([1, N], F32, tag="r2")
        nc.scalar.activation(out=r2, in_=r1a, func=AF.Ln)
        r1b = lnrows.tile([1, N], F32, tag="r1")
        nc.scalar.activation(out=r1b, in_=r2, func=AF.Exp, scale=-0.5)
        arow = lnrows.tile([1, N], F32, tag="arow")
        nc.vector.tensor_mul(arow, r1b, maskrow)
        brow = lnrows.tile([1, N], F32, tag="brow")
        nc.vector._custom_dve(NEGMUL, out=brow, in0=r0, in1=arow)
        ab = bigb.tile([P, N], F32, tag="ab")
        nc.gpsimd.partition_broadcast(out_ap=ab, in_ap=arow)
        bb = bigb.tile([P, N], F32, tag="bb")
        nc.gpsimd.partition_broadcast(out_ap=bb, in_ap=brow)
        last = layer == NLAYERS - 1
        for ct in range(2):
            u1 = epool1.tile([P, N], F32, tag="u1")
            nc.vector.tensor_mul(u1, znew[:, ct, :], ab)
            u2 = epool1.tile([P, N], F32, tag="u2")
            nc.vector.tensor_add(u2, u1, bb)
            nc.scalar.activation(out=xt32[:, ct, :], in_=u2,
                                 func=(AF.Relu if last else AF.Identity),
                                 bias=lnwb[:, 2 + ct:3 + ct],
                                 scale=lnwb[:, ct:ct + 1])

    for ct in range(2):
        nc.sync.dma_start(out=d["out"][ct], in_=xt32[:, ct, :])
    ctx.close()


# --------------------------------------------------------------------------
# host wrapper
# --------------------------------------------------------------------------

_NC_CACHE = {}


def _get_nc():
    if "nc" not in _NC_CACHE:
        _NC_CACHE["nc"] = build_kernel()
    return _NC_CACHE["nc"]


def _prep_shared(W_att, a_att, W_out, a_out, ln_w, ln_b):
    # m1rhs = [Wcat(512) | w1(8) | w2(8)] : [256, 528]
    Wcat = np.transpose(W_att, (1, 0, 2)).reshape(NFEAT, NHEADS * NHID)
    w1 = np.einsum("hfo,ho->fh", W_att, a_att[:, :NHID])
    w2 = np.einsum("hfo,ho->fh", W_att, a_att[:, NHID:])
    m1 = np.concatenate([Wcat, w1, w2], axis=1).astype(np.float32)  # [256, 528]
    m1rhs = m1.reshape(2, 128, 528)
    # wout = [W_out | W_out@ao1 | W_out@ao2] : [512, 258] -> [8, 64, 258]
    b1 = W_out @ a_out[:NCLASS]
    b2 = W_out @ a_out[NCLASS:]
    wo = np.concatenate([W_out, b1[:, None], b2[:, None]], axis=1).astype(BF)
    wout = wo.reshape(NHEADS, 64, 258)
    lnwb = np.stack([ln_w[:128], ln_w[128:], ln_b[:128], ln_b[128:]],
                    axis=1).astype(np.float32)                  # [128, 4]
    return m1rhs, wout, lnwb


def kernel(x, adj, mask, W_att, a_att, W_out, a_out, ln_w, ln_b):
    x = np.asarray(x, np.float32)
    adj = np.asarray(adj)
    mask = np.asarray(mask)
    m1rhs, wout, lnwb = _prep_shared(
        np.asarray(W_att, np.float32), np.asarray(a_att, np.float32),
        np.asarray(W_out, np.float32), np.asarray(a_out, np.float32),
        np.asarray(ln_w, np.float32), np.asarray(ln_b, np.float32))

    in_maps = []
    for b in range(B):
        xt = np.ascontiguousarray(x[b].T).reshape(2, 128, N)
        adjt = (adj[b] > 0).astype(np.float32).T
        ladjt = ((adjt - 1.0) * MASK_NEG).astype(BF).reshape(NJ, 128, N)
        maskrow = (mask[b] != 0).astype(np.float32).reshape(1, N)
        in_maps.append({
            "xt": xt, "ladjt": np.ascontiguousarray(ladjt),
            "maskrow": maskrow, "m1rhs": m1rhs, "wout": wout, "lnwb": lnwb,
        })

    nc = _get_nc()
    res = run_bass_kernel_spmd(nc, in_maps, core_ids=list(range(B)))
    out = np.empty((B, N, NCLASS), np.float32)
    for b in range(B):
        xt_out = res.results[b]["xt_out"].reshape(NFEAT, N)
        out[b] = xt_out.T
    return out, out[:, 0, :]


if __name__ == "__main__":
    rng = np.random.default_rng(0)
    ins = {
        "x": rng.standard_normal((B, N, NFEAT)).astype(np.float32),
        "adj": (rng.random((B, N, N)) < 0.5).astype(np.int32),
        "mask": (rng.random((B, N)) < 0.5).astype(np.int32),
        "W_att": (rng.standard_normal((NHEADS, NFEAT, NHID)) * 0.05).astype(np.float32),
        "a_att": (rng.standard_normal((NHEADS, 2 * NHID)) * 0.05).astype(np.float32),
        "W_out": (rng.standard_normal((NHID * NHEADS, NCLASS)) * 0.05).astype(np.float32),
        "a_out": (rng.standard_normal(2 * NCLASS) * 0.05).astype(np.float32),
        "ln_w": np.ones(NCLASS, np.float32),
        "ln_b": np.zeros(NCLASS, np.float32),
    }
    o1, o2 = kernel(**ins)
    print("out", o1.shape, o1.dtype, float(np.abs(o1).max()))


# revision 28
# speedup vs baseline: 1.2003x; 1.0549x over previous
"""GAT (2-layer, multi-head graph attention) Trainium2 kernel.

Contract: kernel(**inputs) takes the FULL unsharded inputs of
nn_GAT_7421703487704 and returns the full output (tuple matching the
reference: (relu(x), relu(x[:, 0, :]))).

Sharding: data-parallel over batch B=8 -> one graph per NeuronCore (8 cores).
Weights replicated. All shapes hardcoded.

Per-core layout strategy ("transposed" dataflow):
  - Host pre-transposes X -> XT [256,1024] and ships ladjT = (adj^T-1)*88
    (bf16) so the adjacency mask folds additively into attention logits
    pre-exp: exp(lrelu(f1_i+f2_j) + ladj) == adj * exp(lrelu(...)) to ~1e-7.
  - a_att / a_out are folded into the projection matmuls on the host
    (extra output columns f1,f2 / g1,g2).
  - Attention scores are built per [128,1024] tile of P^T [j,i]: the
    lrelu(ladjT + f2_j + f1bcast) runs either as one fused custom-DVE op
    or as two stock scalar_tensor_tensor ops on GpSimd (load balance knob),
    then one ACT Exp.
  - Row-normalisation sums come free as a ones-column in the aggregation
    matmul (stationary [h|1]); softmax division by row-sum is applied
    post-matmul at [64,1024] granularity.
  - Row vectors (f1/g1 rows, 1/rowsum, LN scale/bias rows) are broadcast
    across partitions by bouncing through internal DRAM and re-reading
    with a partition-step-0 access pattern - pure DMA-engine work.
  - LayerNorm runs in transposed layout via ones-matmul column sums;
    rstd uses exp(-0.5*ln(var+eps)) to stay in the exp/ln ACT table set.
"""

import sys

sys.path.insert(0, "/opt/trn_rl_repo")

import numpy as np
import ml_dtypes

import concourse.bass as bass
import concourse.mybir as mybir
import concourse.tile as tile
from concourse import bacc
from concourse.bass import ts
from concourse.bass_utils import run_bass_kernel_spmd
from concourse.masks import make_identity
import concourse.dve_ops as dops
from concourse.dve_ops import DveOp
from concourse.dve_spec import (
    Spec, Src0, Src1, C0, C1, Zero, One, maxx, minn, relu, sq,
    lower as dve_lower, _has_src1,
)
from concourse.dve_uop import DveOpSpec

F32 = mybir.dt.float32
BF16 = mybir.dt.bfloat16
AF = mybir.ActivationFunctionType
ALU = mybir.AluOpType
BF = ml_dtypes.bfloat16

B, N, NFEAT, NHID, NHEADS, NCLASS, NLAYERS = 8, 1024, 256, 64, 8, 256, 2
ALPHA = 0.2
OUTER_SLOPE = 0.01
LN_EPS = 1e-5
MASK_NEG = 88.0  # exp(-88) == 0 in f32; additive mask magnitude
NT = N // 128    # 8 node tiles
NJ = N // 128    # 8 j tiles

# Per-(head,jt) S-tile engine split: tiles with (h*NJ+jt) % GP_MOD < GP_CUT
# run the add+lrelu on GpSimd (2 stock stt ops); the rest use the fused
# custom-DVE op. Tuned from profile engine-occupancy.
GP_CUT = 0
GP_MOD = 9


# --------------------------------------------------------------------------
# custom DVE ops
# --------------------------------------------------------------------------

def _register_op(name, spec, subdim=False):
    if name in dops._SUB_OPCODE_FOR_NAME:
        return next(o for o in dops.OPS if o.name == name)
    row = dops._CUSTOM_DVE_ROW_BASE + len(dops.OPS)
    shas = {}
    for ver in ("v3", "v4"):
        try:
            s = DveOpSpec(name=name, opcode=row, uops=dve_lower(spec, ver=ver),
                          rd1_en=_has_src1(spec))
            shas[ver] = s.sha(ver)
        except Exception:
            pass
    op = DveOp(name, spec, subdim=subdim, uops_sha=shas)
    dops.OPS.append(op)
    dops.CUSTOM_DVE_SPECS[name] = spec
    dops._SUB_OPCODE_FOR_NAME[name] = row
    return op


def _make_ops():
    # t = lrelu((Src0 + C0) + Src1, alpha=C1)
    x = (Src0 + C0) + Src1
    lrelu_add3 = _register_op("GAT_LRELU_ADD3", Spec(
        body=maxx(x, x * C1),
        reference=lambda in0, in1, s0, s1: np.maximum(
            (in0 + s0) + in1, ((in0 + s0) + in1) * s1),
    ))
    # y = relu(Src0) + C0*min(Src1,1) - C0   (== lrelu_C0(elu(z)) given
    # Src0=z, Src1=exp(z); C0=1 gives plain elu(z))
    elu_y = _register_op("GAT_ELU_Y", Spec(
        body=relu(Src0) + minn(Src1, One) * C0 - C0,
        reference=lambda in0, in1, s0, s1: np.maximum(in0, 0)
        + np.minimum(in1, 1.0) * s0 - s0,
    ))
    # var+eps = Src0*C0 - Src1^2 + C1
    var_row = _register_op("GAT_VAR_ROW", Spec(
        body=Src0 * C0 - sq(Src1) + C1,
        reference=lambda in0, in1, s0, s1: in0 * s0 - in1 * in1 + s1,
    ))
    # out = Zero - Src0*Src1
    negmul = _register_op("GAT_NEGMUL", Spec(
        body=Zero - Src0 * Src1,
        reference=lambda in0, in1, s0, s1: -(in0 * in1),
    ))
    return lrelu_add3, elu_y, var_row, negmul


# --------------------------------------------------------------------------
# device kernel
# --------------------------------------------------------------------------

def build_kernel():
    LRELU_ADD3, ELU_Y, VAR_ROW, NEGMUL = _make_ops()

    nc = bacc.Bacc("TRN2", target_bir_lowering=False, debug=False)
    d = {
        "xt": nc.dram_tensor("xt", [2, 128, N], F32, kind="ExternalInput"),
        "ladjt": nc.dram_tensor("ladjt", [NJ, 128, N], BF16, kind="ExternalInput"),
        "maskrow": nc.dram_tensor("maskrow", [1, N], F32, kind="ExternalInput"),
        "m1rhs": nc.dram_tensor("m1rhs", [2, 128, 528], BF16, kind="ExternalInput"),
        "wout": nc.dram_tensor("wout", [NHEADS, 64, 258], BF16, kind="ExternalInput"),
        "lnwb": nc.dram_tensor("lnwb", [128, 4], F32, kind="ExternalInput"),
        "out": nc.dram_tensor("xt_out", [2, 128, N], F32, kind="ExternalOutput"),
    }
    with tile.TileContext(nc) as tc:
        _build_body(nc, tc, d, LRELU_ADD3, ELU_Y, VAR_ROW, NEGMUL)
    nc.compile()
    return nc


def _bcast_src(row_ap, nparts):
    """Partition-step-0 AP replicating a DRAM row across nparts partitions."""
    return bass.AP(tensor=row_ap.tensor, offset=row_ap.offset,
                   ap=[[0, nparts]] + row_ap.ap[1:])


def _build_body(nc, tc, d, LRELU_ADD3, ELU_Y, VAR_ROW, NEGMUL):
    from contextlib import ExitStack
    ctx = ExitStack()
    P = 128

    const = ctx.enter_context(tc.tile_pool(name="const", bufs=1))
    work = ctx.enter_context(tc.tile_pool(name="work", bufs=1))
    dram = ctx.enter_context(tc.tile_pool(name="dram", bufs=1, space="DRAM"))

    # ---- persistent SBUF state ----
    ladjt = const.tile([P, NJ, N], BF16)          # 16KB/part
    for jt in range(NJ):
        nc.sync.dma_start(out=ladjt[:, jt, :], in_=d["ladjt"][jt])
    m1rhs = const.tile([P, 2, 528], BF16)
    for kt in range(2):
        nc.sync.dma_start(out=m1rhs[:, kt, :], in_=d["m1rhs"][kt])
    wout = const.tile([64, NHEADS, 258], BF16)
    for h in range(NHEADS):
        nc.sync.dma_start(out=wout[:, h, :], in_=d["wout"][h])
    lnwb = const.tile([P, 4], F32)
    nc.sync.dma_start(out=lnwb, in_=d["lnwb"][:, :])
    maskrow = const.tile([1, N], F32)
    nc.sync.dma_start(out=maskrow, in_=d["maskrow"][:, :])
    ident = const.tile([P, P], F32)
    make_identity(nc, ident)
    ones_bf = const.tile([P, 1], BF16)
    nc.vector.memset(ones_bf, 1.0)

    xt32 = work.tile([P, 2, N], F32)              # residual state (f32)
    for ct in range(2):
        nc.sync.dma_start(out=xt32[:, ct, :], in_=d["xt"][ct])
    xt16 = work.tile([P, 2, N], BF16)
    for ct in range(2):
        nc.vector.tensor_copy(out=xt16[:, ct, :], in_=xt32[:, ct, :])

    # h-augmented: per head [64 cols | ones]; stationary slices [128, 65]
    haug = work.tile([P, NT, 8 * 65], BF16)
    nc.vector.memset(haug, 1.0)  # ones columns at offsets h*65+64 persist
    hoaug = work.tile([P, NT, 257], BF16)         # [256 | ones]
    nc.vector.memset(hoaug[:, :, 256:257], 1.0)
    fcols = work.tile([P, NT, 16], F32)           # [f1(8) | f2(8)] per n-tile
    gcols = work.tile([P, NT, 2], F32)            # [g1 | g2] per n-tile
    yt = work.tile([64, NHEADS, N], BF16)         # y^T feature-blocks (K=64)
    f1row = work.tile([NHEADS, N], BF16)
    g1row = work.tile([1, N], BF16)
    znew = work.tile([P, 2, N], F32)              # pre-LN residual sum
    zb16 = work.tile([P, 2, N], BF16)
    sqb = work.tile([P, 2, N], BF16)

    # DRAM bounce rows for partition broadcasts
    dr_f1 = dram.tile([NHEADS, N], BF16)
    dr_r = dram.tile([NHEADS, N], F32)
    dr_g1 = dram.tile([1, N], BF16)
    dr_ro = dram.tile([1, N], F32)
    dr_ab = dram.tile([2, N], F32)

    spool = ctx.enter_context(tc.tile_pool(name="spool", bufs=4))
    ppool = ctx.enter_context(tc.tile_pool(name="ppool", bufs=3))
    f1bpool = ctx.enter_context(tc.tile_pool(name="f1b", bufs=2))
    epool = ctx.enter_context(tc.tile_pool(name="epool", bufs=2))
    epool1 = ctx.enter_context(tc.tile_pool(name="epool1", bufs=1))
    lnrows = ctx.enter_context(tc.tile_pool(name="lnrows", bufs=1))
    rowpool = ctx.enter_context(tc.tile_pool(name="rows", bufs=2))
    bigb = ctx.enter_context(tc.tile_pool(name="bigb", bufs=1))

    # Single PSUM pool: 4 rotating slots x 2 banks (4KB/part) = all 8 banks.
    psum = ctx.enter_context(tc.tile_pool(name="ps", bufs=4, space="PSUM"))

    HALF = [slice(0, 512), slice(512, 1024)]

    for layer in range(NLAYERS):
        # ================= M1: [h|f1|f2] = X @ [Wcat|w1|w2] (fp32) ==========
        for nt in range(NT):
            pa = psum.tile([P, 528], F32, tag="ps")
            for kt in range(2):
                nc.tensor.matmul(pa[:, 0:512], lhsT=xt16[:, kt, ts(nt, 128)],
                                 rhs=m1rhs[:, kt, 0:512],
                                 start=(kt == 0), stop=(kt == 1))
                nc.tensor.matmul(pa[:, 512:528], lhsT=xt16[:, kt, ts(nt, 128)],
                                 rhs=m1rhs[:, kt, 512:528],
                                 start=(kt == 0), stop=(kt == 1))
            nc.vector.tensor_copy(
                out=haug[:, nt, :].rearrange("p (h c) -> p h c", c=65)[:, :, 0:64],
                in_=pa[:, 0:512].rearrange("p (h c) -> p h c", c=64))
            nc.vector.tensor_copy(out=fcols[:, nt, :], in_=pa[:, 512:528])

        # ================= f1 rows -> DRAM bounce =================
        pf = psum.tile([8, N], F32, tag="ps")
        for nt in range(NT):
            nc.tensor.transpose(pf[:, ts(nt, 128)], fcols[:, nt, 0:8], ident)
        nc.vector.tensor_copy(out=f1row, in_=pf)

        # ================= inner attention, per head =================
        for h in range(NHEADS):
            f1sh = rowpool.tile([1, N], BF16, tag="f1sh")
            nc.sync.dma_start(out=f1sh, in_=f1row[h:h + 1, :])
            f1b = f1bpool.tile([P, N], BF16, tag="f1b")
            nc.gpsimd.partition_broadcast(out_ap=f1b, in_ap=f1sh)
            ph = psum.tile([65, N], F32, tag="ps")
            for jt in range(NJ):
                f2c = fcols[:, jt, 8 + h:9 + h]
                p_ = ppool.tile([P, N], BF16, tag="p")
                if (h * NJ + jt) % GP_MOD < GP_CUT:
                    ta = spool.tile([P, N], BF16, tag="ta")
                    nc.gpsimd.tensor_add(ta, ladjt[:, jt, :], f1b)
                    tb = spool.tile([P, N], BF16, tag="tb")
                    nc.scalar.activation(out=tb, in_=ta, func=AF.Lrelu,
                                         bias=f2c, alpha=ALPHA)
                    nc.scalar.activation(out=p_, in_=tb, func=AF.Exp)
                else:
                    t0 = spool.tile([P, N], BF16, tag="t")
                    nc.vector._custom_dve(LRELU_ADD3, out=t0,
                                          in0=ladjt[:, jt, :], in1=f1b,
                                          s0=f2c, s1=ALPHA)
                    nc.scalar.activation(out=p_, in_=t0, func=AF.Exp)
                for s in range(2):
                    nc.tensor.matmul(ph[:, HALF[s]],
                                     lhsT=haug[:, jt, h * 65:h * 65 + 65],
                                     rhs=p_[:, HALF[s]],
                                     start=(jt == 0), stop=(jt == NJ - 1))
            # epilogue: y^T[h] = lrelu01(elu(hp / r))
            rsb = rowpool.tile([65, N], F32, tag="rsb")
            nc.scalar.activation(out=rsb[64:65, :], in_=ph[64:65, :],
                                 func=AF.Identity)
            rr0 = rowpool.tile([1, N], F32, tag="rr0")
            nc.sync.dma_start(out=rr0, in_=rsb[64:65, :])
            rinv = rowpool.tile([1, N], F32, tag="rinv")
            nc.vector.reciprocal_approx_fast(out=rinv, in_=rr0)
            rb = epool.tile([64, N], F32, tag="rb")
            nc.gpsimd.partition_broadcast(out_ap=rb, in_ap=rinv)
            z = epool.tile([64, N], BF16, tag="z")
            nc.vector.tensor_mul(z, ph[0:64, :], rb)
            e = epool.tile([64, N], BF16, tag="e")
            nc.scalar.activation(out=e, in_=z, func=AF.Exp)
            nc.vector._custom_dve(ELU_Y, out=yt[:, h, :], in0=z, in1=e,
                                  s0=OUTER_SLOPE)

        # ================= M3: [ho|g1|g2] = y @ [W_out|b1|b2] ===============
        for nt in range(NT):
            pa = psum.tile([P, 258], F32, tag="ps")
            for h in range(NHEADS):
                nc.tensor.matmul(pa[:, 0:258], lhsT=yt[:, h, ts(nt, 128)],
                                 rhs=wout[:, h, 0:258],
                                 start=(h == 0), stop=(h == NHEADS - 1))
            nc.vector.tensor_copy(out=hoaug[:, nt, 0:256], in_=pa[:, 0:256])
            nc.vector.tensor_copy(out=gcols[:, nt, :], in_=pa[:, 256:258])

        # ================= g1 row =================
        pf2 = psum.tile([2, N], F32, tag="ps")
        for nt in range(NT):
            nc.tensor.transpose(pf2[:, ts(nt, 128)], gcols[:, nt, 0:2], ident)
        nc.vector.tensor_copy(out=g1row, in_=pf2[0:1, :])
        g1b = f1bpool.tile([P, N], BF16, tag="f1b")
        nc.gpsimd.partition_broadcast(out_ap=g1b, in_ap=g1row)

        # ================= outer attention =================
        pu0 = psum.tile([P, N], F32, tag="ps")
        pu1 = psum.tile([P, N], F32, tag="ps")
        pro = psum.tile([1, N], F32, tag="ps")
        for jt in range(NJ):
            g2c = gcols[:, jt, 1:2]
            t0 = spool.tile([P, N], BF16, tag="t")
            nc.vector._custom_dve(LRELU_ADD3, out=t0, in0=ladjt[:, jt, :],
                                  in1=g1b, s0=g2c, s1=ALPHA)
            po = ppool.tile([P, N], BF16, tag="p")
            nc.scalar.activation(out=po, in_=t0, func=AF.Exp)
            for s in range(2):
                nc.tensor.matmul(pu0[:, HALF[s]], lhsT=hoaug[:, jt, 0:128],
                                 rhs=po[:, HALF[s]],
                                 start=(jt == 0), stop=(jt == NJ - 1))
                nc.tensor.matmul(pu1[:, HALF[s]], lhsT=hoaug[:, jt, 128:256],
                                 rhs=po[:, HALF[s]],
                                 start=(jt == 0), stop=(jt == NJ - 1))
                nc.tensor.matmul(pro[:, HALF[s]], lhsT=hoaug[:, jt, 256:257],
                                 rhs=po[:, HALF[s]],
                                 start=(jt == 0), stop=(jt == NJ - 1))
        # z = uoT / ro ; znew = xt + elu(z)
        roinv = rowpool.tile([1, N], F32, tag="roinv")
        nc.vector.reciprocal_approx_fast(out=roinv, in_=pro[0:1, :])
        rob = bigb.tile([P, N], F32, tag="rob")
        nc.gpsimd.partition_broadcast(out_ap=rob, in_ap=roinv)
        for ct, pu in enumerate((pu0, pu1)):
            zo = epool1.tile([P, N], BF16, tag="zo")
            nc.vector.tensor_mul(zo, pu, rob)
            eo = epool1.tile([P, N], BF16, tag="eo")
            nc.scalar.activation(out=eo, in_=zo, func=AF.Exp)
            t1 = epool1.tile([P, N], BF16, tag="t1o")
            nc.vector._custom_dve(ELU_Y, out=t1, in0=zo, in1=eo, s0=1.0)
            nc.vector.tensor_add(znew[:, ct, :], xt32[:, ct, :], t1)

        # ================= masked LayerNorm (transposed) =================
        for ct in range(2):
            nc.vector.tensor_copy(out=zb16[:, ct, :], in_=znew[:, ct, :])
            nc.vector.tensor_mul(sqb[:, ct, :], zb16[:, ct, :], zb16[:, ct, :])
        pSs = psum.tile([1, N], F32, tag="ps")
        pSq = psum.tile([1, N], F32, tag="ps")
        for s in range(2):
            for ct in range(2):
                nc.tensor.matmul(pSs[:, HALF[s]], lhsT=ones_bf,
                                 rhs=zb16[:, ct, HALF[s]],
                                 start=(ct == 0), stop=(ct == 1))
                nc.tensor.matmul(pSq[:, HALF[s]], lhsT=ones_bf,
                                 rhs=sqb[:, ct, HALF[s]],
                                 start=(ct == 0), stop=(ct == 1))
        # rows: mu, var+eps, rstd = exp(-0.5*ln(var+eps)), A = rstd*mask, B=-mu*A
        r0 = lnrows.tile([1, N], F32, tag="r0")
        nc.vector.tensor_scalar_mul(r0, pSs[0:1, :], 1.0 / NFEAT)
        r1a = lnrows.tile([1, N], F32, tag="r1")
        nc.vector._custom_dve(VAR_ROW, out=r1a, in0=pSq[0:1, :], in1=r0,
                              s0=1.0 / NFEAT, s1=LN_EPS)
        r2 = lnrows.tile([1, N], F32, tag="r2")
        nc.scalar.activation(out=r2, in_=r1a, func=AF.Ln)
        r1b = lnrows.tile([1, N], F32, tag="r1")
        nc.scalar.activation(out=r1b, in_=r2, func=AF.Exp, scale=-0.5)
        arow = lnrows.tile([1, N], F32, tag="arow")
        nc.vector.tensor_mul(arow, r1b, maskrow)
        brow = lnrows.tile([1, N], F32, tag="brow")
        nc.vector._custom_dve(NEGMUL, out=brow, in0=r0, in1=arow)
        ab = bigb.tile([P, N], F32, tag="ab")
        nc.gpsimd.partition_broadcast(out_ap=ab, in_ap=arow)
        bb = bigb.tile([P, N], F32, tag="bb")
        nc.gpsimd.partition_broadcast(out_ap=bb, in_ap=brow)
        last = layer == NLAYERS - 1
        for ct in range(2):
            u1 = epool1.tile([P, N], F32, tag="u1")
            nc.vector.tensor_mul(u1, znew[:, ct, :], ab)
            u2 = epool1.tile([P, N], F32, tag="u2")
            nc.vector.tensor_add(u2, u1, bb)
            nc.scalar.activation(out=xt32[:, ct, :], in_=u2,
                                 func=(AF.Relu if last else AF.Identity),
                                 bias=lnwb[:, 2 + ct:3 + ct],
                                 scale=lnwb[:, ct:ct + 1])
            if not last:
                nc.vector.tensor_copy(out=xt16[:, ct, :], in_=xt32[:, ct, :])

    for ct in range(2):
        nc.sync.dma_start(out=d["out"][ct], in_=xt32[:, ct, :])
    ctx.close()


# --------------------------------------------------------------------------
# host wrapper
# --------------------------------------------------------------------------

_NC_CACHE = {}


def _get_nc():
    if "nc" not in _NC_CACHE:
        _NC_CACHE["nc"] = build_kernel()
    return _NC_CACHE["nc"]


def _prep_shared(W_att, a_att, W_out, a_out, ln_w, ln_b):
    # m1rhs = [Wcat(512) | w1(8) | w2(8)] : [256, 528]
    Wcat = np.transpose(W_att, (1, 0, 2)).reshape(NFEAT, NHEADS * NHID)
    w1 = np.einsum("hfo,ho->fh", W_att, a_att[:, :NHID])
    w2 = np.einsum("hfo,ho->fh", W_att, a_att[:, NHID:])
    m1 = np.concatenate([Wcat, w1, w2], axis=1).astype(BF)  # [256, 528]
    m1rhs = m1.reshape(2, 128, 528)
    # wout = [W_out | W_out@ao1 | W_out@ao2] : [512, 258] -> [8, 64, 258]
    b1 = W_out @ a_out[:NCLASS]
    b2 = W_out @ a_out[NCLASS:]
    wo = np.concatenate([W_out, b1[:, None], b2[:, None]], axis=1).astype(BF)
    wout = wo.reshape(NHEADS, 64, 258)
    lnwb = np.stack([ln_w[:128], ln_w[128:], ln_b[:128], ln_b[128:]],
                    axis=1).astype(np.float32)                  # [128, 4]
    return m1rhs, wout, lnwb


def kernel(x, adj, mask, W_att, a_att, W_out, a_out, ln_w, ln_b):
    x = np.asarray(x, np.float32)
    adj = np.asarray(adj)
    mask = np.asarray(mask)
    m1rhs, wout, lnwb = _prep_shared(
        np.asarray(W_att, np.float32), np.asarray(a_att, np.float32),
        np.asarray(W_out, np.float32), np.asarray(a_out, np.float32),
        np.asarray(ln_w, np.float32), np.asarray(ln_b, np.float32))

    in_maps = []
    for b in range(B):
        xt = np.ascontiguousarray(x[b].T).reshape(2, 128, N)
        adjt = (adj[b] > 0).astype(np.float32).T
        ladjt = ((adjt - 1.0) * MASK_NEG).astype(BF).reshape(NJ, 128, N)
        maskrow = (mask[b] != 0).astype(np.float32).reshape(1, N)
        in_maps.append({
            "xt": xt, "ladjt": np.ascontiguousarray(ladjt),
            "maskrow": maskrow, "m1rhs": m1rhs, "wout": wout, "lnwb": lnwb,
        })

    nc = _get_nc()
    res = run_bass_kernel_spmd(nc, in_maps, core_ids=list(range(B)))
    out = np.empty((B, N, NCLASS), np.float32)
    for b in range(B):
        xt_out = res.results[b]["xt_out"].reshape(NFEAT, N)
        out[b] = xt_out.T
    return out, out[:, 0, :]


if __name__ == "__main__":
    rng = np.random.default_rng(0)
    ins = {
        "x": rng.standard_normal((B, N, NFEAT)).astype(np.float32),
        "adj": (rng.random((B, N, N)) < 0.5).astype(np.int32),
        "mask": (rng.random((B, N)) < 0.5).astype(np.int32),
        "W_att": (rng.standard_normal((NHEADS, NFEAT, NHID)) * 0.05).astype(np.float32),
        "a_att": (rng.standard_normal((NHEADS, 2 * NHID)) * 0.05).astype(np.float32),
        "W_out": (rng.standard_normal((NHID * NHEADS, NCLASS)) * 0.05).astype(np.float32),
        "a_out": (rng.standard_normal(2 * NCLASS) * 0.05).astype(np.float32),
        "ln_w": np.ones(NCLASS, np.float32),
        "ln_b": np.zeros(NCLASS, np.float32),
    }
    o1, o2 = kernel(**ins)
    print("out", o1.shape, o1.dtype, float(np.abs(o1).max()))
